# revision 10
# baseline (speedup 1.0000x reference)
"""Trainium2 Bass kernel for BiDAF-style bidirectional attention.

Reference computation (per batch element n; M=1 folded away):
    s[i,j]  = h[i].w_h + u[j].w_u + (h[i]*u[j]).w_hu + b      [JX, JQ]
    a_u     = softmax_j(s);     u_a[i] = sum_j a_u[i,j] u[j]   (c2q)
    a_h     = softmax_i(max_j s);  h_a = sum_i a_h[i] h[i]     (q2c)
    out     = concat(h, u_a, h*u_a, h*h_a)                     [JX, 4D]

Sharding: data-parallel over batch N=8, one NeuronCore per batch element.
alpha_b drops out entirely (both softmaxes are shift-invariant).

v3 structure (51us f32 baseline -> target ~30us):
  - output split in DRAM: out0 = f32 [JX, D] passthrough column, written by
    a single DRAM->DRAM DMA of h issued at kernel start (flows during the
    otherwise-idle init window, no SBUF round trip, no cast); out123 = bf16
    [JX, 3D] for the three computed columns (store traffic 6MB -> 3MB, the
    2e-2 rel-err gate leaves ~5x margin over bf16 rounding).
  - bf16 matmul operands (hT, ET, u, uw', m_exp): PE moving cost 1 cyc/col
    instead of 1.5 (f32r), PSUM->SBUF evictions cast-on-copy to bf16.
    All accumulation stays f32 in PSUM.
  - few big DMAs (~0.6us issue + ~0.9us completion each): h loads as 2
    singles + 3 pair-tiles, output writes as 6 merged 4-tile transfers,
    all on the sync queue in readiness order.
  - PE spine: block-0 transposes -> scores-b0 -> block-1 transposes ->
    etr-b0 -> scores-b1 -> etr-b1 -> c2q/hap interleaved, so EXP-1 lands
    ~10us earlier than the baseline's schedule.
  - tail elementwise ops are 2-tile-wide (1024 cols) where legal to
    amortize the ~400ns per-instruction engine overhead; col1 (the only
    PSUM-reading mul with a per-tile scalar) stays per-tile, split
    ACT/DVE.  GpSimd (no PSUM port, ~1.6x slower) gets SBUF-only pairs.
"""

import numpy as np

N_B, M_B, JX, JQ, D = 8, 1, 1024, 128, 512
P = 128
NT = JX // P   # 8 i-tiles
KC = D // P    # 4 d-chunks
IB = 512       # i-block width for score matmuls
NB = JX // IB  # 2 blocks
TPB = NT // NB  # tiles per block

_CACHE = {}


def _build_program():
    from contextlib import ExitStack

    import concourse.bass as bass
    import concourse.tile as tile
    from concourse import bacc, mybir
    from concourse.masks import make_identity

    f32 = mybir.dt.float32
    f32r = mybir.dt.float32r
    bf16 = mybir.dt.bfloat16
    EXP = mybir.ActivationFunctionType.Exp
    AX = mybir.AxisListType.X
    MUL = mybir.AluOpType.mult
    ds = bass.ds

    nc = bacc.Bacc("TRN2", target_bir_lowering=False, debug=False, num_devices=8)
    h_d = nc.dram_tensor("h", [JX, D], f32, kind="ExternalInput").ap()
    u_d = nc.dram_tensor("u", [JQ, D], f32, kind="ExternalInput").ap()
    aw_d = nc.dram_tensor("alpha_w", [3 * D], f32, kind="ExternalInput").ap()
    out0_d = nc.dram_tensor("out0", [JX, D], f32, kind="ExternalOutput").ap()
    out123_d = nc.dram_tensor("out123", [JX, 3 * D], bf16, kind="ExternalOutput").ap()

    with tile.TileContext(nc) as tc, ExitStack() as ctx:
        consts = ctx.enter_context(tc.tile_pool(name="consts", bufs=1))
        stage = ctx.enter_context(tc.tile_pool(name="stage", bufs=4))
        # PSUM budget (8 banks): tp=2, s0=2, ua=2, acc=1, hap=1
        ps = ctx.enter_context(tc.tile_pool(name="ps", bufs=2, space="PSUM"))

        # ---- PE warmup: f32r matmuls depending only on DVE ops, emitted
        # first so the HAM clock-gate opens (1.2 -> 2.4 GHz) while the h
        # DMAs stream in.
        warm_f = consts.tile([P, D], f32)
        nc.vector.memset(warm_f[:], 0.25)
        warm = consts.tile([P, D], f32r)
        nc.vector.tensor_copy(warm[:], warm_f[:])
        wp = ps.tile([P, D], f32, tag="acc", bufs=1)
        for w in range(2):
            nc.tensor.matmul(
                wp[:], warm[:, ds(0, P)], warm[:], start=True, stop=True,
            )

        # ---- constants ----
        ident_f = consts.tile([P, P], f32)
        make_identity(nc, ident_f[:])
        ident = consts.tile([P, P], f32r)
        nc.vector.tensor_copy(ident[:], ident_f[:])
        ident16 = consts.tile([P, P], bf16)
        nc.vector.tensor_copy(ident16[:], ident_f[:])
        ones_row_f = consts.tile([1, P], f32)
        nc.vector.memset(ones_row_f[:], 1.0)
        ones_row = consts.tile([1, P], f32r)
        nc.scalar.copy(ones_row[:], ones_row_f[:])
        ones_row16 = consts.tile([1, P], bf16)
        nc.scalar.copy(ones_row16[:], ones_row_f[:])
        ones_col = consts.tile([P, 1], f32)
        nc.vector.memset(ones_col[:], 1.0)

        # ---- loads (sync queue): u + aw first (they feed the uw' prep
        # chain), then h tiles 0,1 single (they gate the first transposes)
        # and 2-7 as pairs.
        h_all = consts.tile([P, NT * D], f32r)    # tile t: h[t*128+p, d]
        h_f = h_all[:].bitcast(f32)
        for t in range(2):
            nc.sync.dma_start(
                h_all[:, ds(t * D, D)], h_d[ds(t * P, P), :].bitcast(f32r)
            )
        aw_sb = consts.tile([1, 3 * D], f32r)
        nc.sync.dma_start(aw_sb[:], aw_d.rearrange("(o d) -> o d", o=1).bitcast(f32r))
        u_sb = consts.tile([JQ, D], f32r)
        nc.sync.dma_start(u_sb[:], u_d[:].bitcast(f32r))
        u_f = u_sb[:].bitcast(f32)
        for q in range(3):
            t0 = 2 + 2 * q
            nc.sync.dma_start(
                h_all[:, ds(t0 * D, 2 * D)].rearrange("p (t d) -> p t d", t=2),
                h_d[ds(t0 * P, 2 * P), :]
                .rearrange("(t p) d -> p t d", p=P).bitcast(f32r),
            )

        # passthrough column: DRAM->DRAM copy of h, issued after the loads so
        # its descriptors don't block them; reshaped to 8KB rows so the SDMA
        # engines interleave it with later traffic at packet granularity.
        nc.sync.dma_start(
            out0_d.rearrange("(a b) d -> a (b d)", b=4),
            h_d.rearrange("(a b) d -> a (b d)", b=4),
        )

        u16 = consts.tile([JQ, D], bf16)
        nc.gpsimd.tensor_copy(u16[:], u_f)

        # alpha_w partition-broadcast on-chip: K=1 matmuls into s0 PSUM,
        # read directly by the DVE.  whu first (gates uw').
        def wcast(c):
            wt = ps.tile([P, D], f32, tag="s0")
            nc.tensor.matmul(
                wt[:], ones_row[:], aw_sb[:, ds(c * D, D)], start=True, stop=True
            )
            return wt

        # ---- hT via PE transposes (f32r moving ident), cast-evict to bf16.
        hT16 = consts.tile([P, KC * JX], bf16)    # chunk k: hT[k*128+p, i]
        hT3 = hT16[:].rearrange("p (k x) -> p k x", k=KC)

        def transpose_tile(t):
            pt = ps.tile([P, KC * P], f32r, tag="tp")
            for k in range(KC):
                nc.tensor.transpose(
                    pt[:, ds(k * P, P)], h_all[:, ds(t * D + k * P, P)], ident[:]
                )
            ev = nc.vector.tensor_copy if t % 2 == 0 else nc.scalar.copy
            ev(hT3[:, :, ds(t * P, P)],
               pt[:].bitcast(f32).rearrange("p (k x) -> p k x", k=KC))

        transpose_tile(0)
        whu_p = wcast(2)
        wh_p = wcast(0)
        transpose_tile(1)
        wu_p = wcast(1)

        # uw[j,d] = u[j,d]*w_hu[d] + w_h[d];  uwu[j] = sum_d u[j,d]*w_u[d]
        uw = consts.tile([JQ, D], f32r)
        uw0 = consts.tile([JQ, D], f32)
        nc.vector.tensor_mul(uw0[:], u_f, whu_p[:])
        nc.vector.tensor_add(uw[:], uw0[:], wh_p[:])
        uwtmp = consts.tile([JQ, D], f32)
        uwu = consts.tile([JQ, 1], f32)
        nc.vector.scalar_tensor_tensor(
            uwtmp[:], u_f, 1.0, wu_p[:],
            op0=MUL, op1=MUL, accum_out=uwu[:],
        )

        transpose_tile(2)

        # uwT16[d_chunk][j]: 4 transposes into one PSUM bank, one cast-evict
        uwT16 = consts.tile([P, KC * JQ], bf16)
        ptw = ps.tile([P, KC * P], f32r, tag="tp")
        for k in range(KC):
            nc.tensor.transpose(ptw[:, ds(k * P, P)], uw[:, ds(k * P, P)], ident[:])
        nc.scalar.copy(uwT16[:], ptw[:].bitcast(f32))

        transpose_tile(3)

        # ---- scores (transposed layout): sT[j,i] over a 512-wide i-block
        ET16 = consts.tile([JQ, JX], bf16)        # exp(sT + uwu[j]) = exp(s - b)
        m16 = consts.tile([P, NT], f32r)          # per i-tile: max_j ET (f32r: hap pairs with f32r h_all)
        z_rec = consts.tile([P, NT], f32)         # per i-tile: 1/sum_j ET

        def block_scores(b):
            sp = ps.tile([JQ, IB], f32, tag="s0")
            for k in range(KC):
                nc.tensor.matmul(
                    sp[:], uwT16[:, ds(k * JQ, JQ)], hT3[:, k, ds(b * IB, IB)],
                    start=(k == 0), stop=(k == KC - 1),
                )
            # ET = exp(sT + uwu[j]); uwu is the per-partition (j) ACT bias
            nc.scalar.activation(ET16[:, ds(b * IB, IB)], sp[:], EXP, bias=uwu[:])

        mrow = consts.tile([P, 1], f32)

        def block_reduce(b):
            # re-transpose ET (4 tiles into one bank, bf16); batched reduces.
            # For block 1, mrow (the q2c chain head) is squeezed between MAX
            # and the zsum so the hap/bc path isn't queued behind it.
            et = ps.tile([P, TPB * P], bf16, tag="tp")
            for q in range(TPB):
                t = b * TPB + q
                nc.tensor.transpose(
                    et[:, ds(q * P, P)], ET16[:, ds(t * P, P)], ident16[:]
                )
            et3 = et[:].rearrange("p (q x) -> p q x", q=TPB)
            nc.vector.reduce_max(m16[:, ds(b * TPB, TPB)], et3, axis=AX)
            if b == 1:
                nc.vector.reduce_sum(mrow[:], m16[:].bitcast(f32), axis=AX)
            zsum = stage.tile([P, TPB], f32, tag="zs")
            nc.vector.reduce_sum(zsum[:], et3, axis=AX)
            nc.vector.reciprocal(z_rec[:, ds(b * TPB, TPB)], zsum[:])

        block_scores(0)
        transpose_tile(4)
        transpose_tile(5)
        transpose_tile(6)
        transpose_tile(7)
        block_reduce(0)
        block_scores(1)

        hap = ps.tile([1, D], f32, tag="hap", bufs=1)

        def hap_block(b):
            # q2c accumulation (single PSUM group spanning both blocks)
            for q in range(TPB):
                t = b * TPB + q
                nc.tensor.matmul(
                    hap[:], m16[:, ds(t, 1)], h_all[:, ds(t * D, D)],
                    start=(b == 0 and q == 0), stop=(b == NB - 1 and q == TPB - 1),
                    skip_group_check=True,
                )

        # ---- c2q: up = unnormalized a_u @ u per tile; col1 = up*zr (per-tile
        # scalar, ACT evens / DVE odds), col2 = col1*h as 2-wide SBUF muls.
        stgA = consts.tile([P, NT * D], bf16)   # u_a        (out123 col 0)
        stgB = consts.tile([P, NT * D], bf16)   # h * u_a    (out123 col 1)
        o4_16 = consts.tile([P, NT * D], bf16)  # h * h_a    (out123 col 2)

        ups = {}

        def c2q_tile(t):
            up = ps.tile([P, D], f32, tag="ua")
            ups[t] = up
            nc.tensor.matmul(
                up[:], ET16[:, ds(t * P, P)], u16[:], start=True, stop=True
            )
            # col1 = up * zr on ACT (frees DVE for col2/o4)
            nc.scalar.mul(stgA[:, ds(t * D, D)], ups[t][:], z_rec[:, ds(t, 1)])

        def col2_tile(t):
            # col2 = (up * zr) * h as one f32-in DVE stt (mixed bf16/f32
            # tensor_tensor on DVE hits a 4x-slow path; this doesn't)
            nc.vector.scalar_tensor_tensor(
                stgB[:, ds(t * D, D)], ups[t][:], z_rec[:, ds(t, 1)],
                h_f[:, ds(t * D, D)], op0=MUL, op1=MUL,
            )

        def col2_pair_gps(t0):
            # GpSimd 2-wide from SBUF (needs col1 done for both tiles)
            nc.gpsimd.tensor_mul(
                stgB[:, ds(t0 * D, 2 * D)],
                stgA[:, ds(t0 * D, 2 * D)],
                h_f[:, ds(t0 * D, 2 * D)],
            )

        def stg_writes(half):
            t0 = half * TPB
            for col, src in ((0, stgA), (1, stgB)):
                nc.sync.dma_start(
                    out123_d[ds(t0 * P, TPB * P), ds(col * D, D)]
                    .rearrange("(t p) c -> p t c", p=P),
                    src[:, ds(t0 * D, TPB * D)].rearrange("p (t c) -> p t c", t=TPB),
                )

        # ---- spine: c2q t0-3 in the EXP-1/reduce-1 shadow; hap prioritized
        # on PE; reduces-b1 + mrow + rzq emitted on DVE before the bulk col2
        # muls so the q2c chain isn't head-of-line blocked.
        c2q_tile(0)
        c2q_tile(1)
        block_reduce(1)
        col2_pair_gps(0)
        c2q_tile(2)
        c2q_tile(3)
        col2_pair_gps(2)
        hap_block(0)
        hap_block(1)

        # q2c normalization chain: rzq = 1/sum_i m_exp[i] folds into the
        # ha_row eviction as an ACT scale, so bc is the NORMALIZED h_a
        # broadcast and o4 is a plain tensor_mul.
        zqp = ps.tile([1, 1], f32, tag="acc", bufs=1)
        nc.tensor.matmul(zqp[:], mrow[:], ones_col[:], start=True, stop=True)
        rzq = consts.tile([1, 1], f32)
        nc.vector.reciprocal(rzq[:], zqp[:])
        ha_row = consts.tile([1, D], bf16)
        nc.scalar.mul(ha_row[:], hap[:], rzq[:])
        bc = ps.tile([P, D], f32, tag="acc", bufs=1)
        nc.tensor.matmul(bc[:], ones_row16[:], ha_row[:], start=True, stop=True)
        bc_sb = consts.tile([P, D], bf16)
        nc.scalar.copy(bc_sb[:], bc[:])

        bc2 = bc[:].rearrange("p (o d) -> p o d", o=1).broadcast_to([P, 2, D])
        bc_sb2 = bc_sb[:].rearrange("p (o d) -> p o d", o=1).broadcast_to([P, 2, D])

        def o4_pair(t0, eng):
            src = bc2 if eng is nc.vector else bc_sb2
            eng.tensor_mul(
                o4_16[:, ds(t0 * D, 2 * D)].rearrange("p (t d) -> p t d", t=2),
                h_f[:, ds(t0 * D, 2 * D)].rearrange("p (t d) -> p t d", t=2),
                src,
            )

        def o4_write(half):
            t0 = half * TPB
            nc.sync.dma_start(
                out123_d[ds(t0 * P, TPB * P), ds(2 * D, D)]
                .rearrange("(t p) c -> p t c", p=P),
                o4_16[:, ds(t0 * D, TPB * D)].rearrange("p (t c) -> p t c", t=TPB),
            )

        c2q_tile(4)
        c2q_tile(5)
        stg_writes(0)
        col2_tile(4)
        c2q_tile(6)
        col2_tile(5)
        c2q_tile(7)
        col2_tile(6)
        col2_tile(7)
        o4_pair(0, nc.vector)
        o4_pair(4, nc.gpsimd)
        o4_pair(2, nc.vector)
        o4_write(0)
        stg_writes(1)
        o4_pair(6, nc.vector)
        nc.sync.dma_start(
            out123_d[ds(4 * P, 2 * P), ds(2 * D, D)]
            .rearrange("(t p) c -> p t c", p=P),
            o4_16[:, ds(4 * D, 2 * D)].rearrange("p (t c) -> p t c", t=2),
        )
        nc.sync.dma_start(
            out123_d[ds(6 * P, 2 * P), ds(2 * D, D)]
            .rearrange("(t p) c -> p t c", p=P),
            o4_16[:, ds(6 * D, 2 * D)].rearrange("p (t c) -> p t c", t=2),
        )

    nc.compile()
    return nc


def _get_nc():
    if "nc" not in _CACHE:
        _CACHE["nc"] = _build_program()
    return _CACHE["nc"]


def _ensure_axon_hooks_stub():
    # concourse imports antenv.axon_hooks when tracing is requested via env;
    # provide a no-op stub if the image lacks it so runs degrade gracefully.
    import sys
    import types

    try:
        import antenv.axon_hooks  # noqa: F401
    except ImportError:
        mod = types.ModuleType("antenv.axon_hooks")
        _hook = [None]
        mod.set_axon_ntff_profile_hook = lambda hook: _hook.__setitem__(0, hook)
        mod.get_axon_ntff_profile_hook = lambda: _hook[0]
        sys.modules["antenv.axon_hooks"] = mod


def _postprocess(res):
    out = np.empty((N_B, JX, 4 * D), dtype=np.float32)
    for n in range(N_B):
        out[n, :, :D] = np.asarray(res.results[n]["out0"])
        out[n, :, D:] = np.asarray(res.results[n]["out123"]).astype(np.float32)
    return out.reshape(N_B, M_B, JX, 4 * D)


def kernel(h, u, alpha_w, alpha_b=None, **_unused):
    _ensure_axon_hooks_stub()
    from concourse.bass_utils import run_bass_kernel_spmd

    h = np.ascontiguousarray(np.asarray(h, dtype=np.float32)).reshape(N_B, JX, D)
    u = np.ascontiguousarray(np.asarray(u, dtype=np.float32)).reshape(N_B, JQ, D)
    alpha_w = np.ascontiguousarray(np.asarray(alpha_w, dtype=np.float32)).reshape(3 * D)

    nc = _get_nc()
    in_maps = [
        {"h": h[n], "u": u[n], "alpha_w": alpha_w} for n in range(N_B)
    ]
    res = run_bass_kernel_spmd(nc, in_maps, core_ids=list(range(N_B)))
    return _postprocess(res)


# revision 11
# speedup vs baseline: 1.0548x; 1.0548x over previous
"""Trainium2 Bass kernel for BiDAF-style bidirectional attention.

Reference computation (per batch element n; M=1 folded away):
    s[i,j]  = h[i].w_h + u[j].w_u + (h[i]*u[j]).w_hu + b      [JX, JQ]
    a_u     = softmax_j(s);     u_a[i] = sum_j a_u[i,j] u[j]   (c2q)
    a_h     = softmax_i(max_j s);  h_a = sum_i a_h[i] h[i]     (q2c)
    out     = concat(h, u_a, h*u_a, h*h_a)                     [JX, 4D]

Sharding: data-parallel over batch N=8, one NeuronCore per batch element.
alpha_b drops out entirely (both softmaxes are shift-invariant).

v3 structure (51us f32 baseline -> target ~30us):
  - output split in DRAM: out0 = f32 [JX, D] passthrough column, written by
    a single DRAM->DRAM DMA of h issued at kernel start (flows during the
    otherwise-idle init window, no SBUF round trip, no cast); out123 = bf16
    [JX, 3D] for the three computed columns (store traffic 6MB -> 3MB, the
    2e-2 rel-err gate leaves ~5x margin over bf16 rounding).
  - bf16 matmul operands (hT, ET, u, uw', m_exp): PE moving cost 1 cyc/col
    instead of 1.5 (f32r), PSUM->SBUF evictions cast-on-copy to bf16.
    All accumulation stays f32 in PSUM.
  - few big DMAs (~0.6us issue + ~0.9us completion each): h loads as 2
    singles + 3 pair-tiles, output writes as 6 merged 4-tile transfers,
    all on the sync queue in readiness order.
  - PE spine: block-0 transposes -> scores-b0 -> block-1 transposes ->
    etr-b0 -> scores-b1 -> etr-b1 -> c2q/hap interleaved, so EXP-1 lands
    ~10us earlier than the baseline's schedule.
  - tail elementwise ops are 2-tile-wide (1024 cols) where legal to
    amortize the ~400ns per-instruction engine overhead; col1 (the only
    PSUM-reading mul with a per-tile scalar) stays per-tile, split
    ACT/DVE.  GpSimd (no PSUM port, ~1.6x slower) gets SBUF-only pairs.
"""

import numpy as np

N_B, M_B, JX, JQ, D = 8, 1, 1024, 128, 512
P = 128
NT = JX // P   # 8 i-tiles
KC = D // P    # 4 d-chunks
IB = 512       # i-block width for score matmuls
NB = JX // IB  # 2 blocks
TPB = NT // NB  # tiles per block

_CACHE = {}


def _build_program():
    from contextlib import ExitStack

    import concourse.bass as bass
    import concourse.tile as tile
    from concourse import bacc, mybir
    from concourse.masks import make_identity
    from concourse.tile_rust import add_dep_helper

    f32 = mybir.dt.float32
    f32r = mybir.dt.float32r
    bf16 = mybir.dt.bfloat16
    EXP = mybir.ActivationFunctionType.Exp
    AX = mybir.AxisListType.X
    MUL = mybir.AluOpType.mult
    ds = bass.ds

    nc = bacc.Bacc("TRN2", target_bir_lowering=False, debug=False, num_devices=8)
    h_d = nc.dram_tensor("h", [JX, D], f32, kind="ExternalInput").ap()
    u_d = nc.dram_tensor("u", [JQ, D], f32, kind="ExternalInput").ap()
    aw_d = nc.dram_tensor("alpha_w", [3 * D], f32, kind="ExternalInput").ap()
    out0_d = nc.dram_tensor("out0", [JX, D], f32, kind="ExternalOutput").ap()
    out123_d = nc.dram_tensor("out123", [JX, 3 * D], bf16, kind="ExternalOutput").ap()

    with tile.TileContext(nc) as tc, ExitStack() as ctx:
        consts = ctx.enter_context(tc.tile_pool(name="consts", bufs=1))
        stage = ctx.enter_context(tc.tile_pool(name="stage", bufs=4))
        # PSUM budget (8 banks): tp=2, s0=2, ua=2, acc=1, hap=1
        ps = ctx.enter_context(tc.tile_pool(name="ps", bufs=2, space="PSUM"))

        # ---- PE warmup: f32r matmuls depending only on DVE ops, emitted
        # first so the HAM clock-gate opens (1.2 -> 2.4 GHz) while the h
        # DMAs stream in.
        warm_f = consts.tile([P, D], f32)
        nc.vector.memset(warm_f[:], 0.25)
        warm = consts.tile([P, D], f32r)
        nc.vector.tensor_copy(warm[:], warm_f[:])
        wp = ps.tile([P, D], f32, tag="acc", bufs=1)
        for w in range(2):
            nc.tensor.matmul(
                wp[:], warm[:, ds(0, P)], warm[:], start=True, stop=True,
            )

        # ---- constants ----
        ident_f = consts.tile([P, P], f32)
        make_identity(nc, ident_f[:])
        ident = consts.tile([P, P], f32r)
        nc.vector.tensor_copy(ident[:], ident_f[:])
        ident16 = consts.tile([P, P], bf16)
        nc.vector.tensor_copy(ident16[:], ident_f[:])
        ones_row_f = consts.tile([1, P], f32)
        nc.vector.memset(ones_row_f[:], 1.0)
        ones_row = consts.tile([1, P], f32r)
        nc.scalar.copy(ones_row[:], ones_row_f[:])
        ones_row16 = consts.tile([1, P], bf16)
        nc.scalar.copy(ones_row16[:], ones_row_f[:])
        ones_col = consts.tile([P, 1], f32)
        nc.vector.memset(ones_col[:], 1.0)

        # ---- loads (sync queue): u + aw first (they feed the uw' prep
        # chain), then h tiles 0,1 single (they gate the first transposes)
        # and 2-7 as pairs.
        h_all = consts.tile([P, NT * D], f32r)    # tile t: h[t*128+p, d]
        h_f = h_all[:].bitcast(f32)
        for t in range(2):
            nc.sync.dma_start(
                h_all[:, ds(t * D, D)], h_d[ds(t * P, P), :].bitcast(f32r)
            )
        aw_sb = consts.tile([1, 3 * D], f32r)
        nc.sync.dma_start(aw_sb[:], aw_d.rearrange("(o d) -> o d", o=1).bitcast(f32r))
        u_sb = consts.tile([JQ, D], f32r)
        nc.sync.dma_start(u_sb[:], u_d[:].bitcast(f32r))
        u_f = u_sb[:].bitcast(f32)
        last_h = None
        for q in range(3):
            t0 = 2 + 2 * q
            last_h = nc.sync.dma_start(
                h_all[:, ds(t0 * D, 2 * D)].rearrange("p (t d) -> p t d", t=2),
                h_d[ds(t0 * P, 2 * P), :]
                .rearrange("(t p) d -> p t d", p=P).bitcast(f32r),
            )

        # passthrough column: DRAM->DRAM copy of h in 8KB rows.  Gated on the
        # last h load: its big descriptors win the per-packet round-robin, so
        # letting it overlap the loads starves them (~150GB/s effective).  In
        # the post-load lull it's free.
        d2d = nc.sync.dma_start(
            out0_d.rearrange("(a b) d -> a (b d)", b=4),
            h_d.rearrange("(a b) d -> a (b d)", b=4),
        )
        add_dep_helper(d2d.ins, last_h.ins, sync=True,
                       reason="keep d2d pass copy out of the load window")

        u16 = consts.tile([JQ, D], bf16)
        nc.gpsimd.tensor_copy(u16[:], u_f)

        # alpha_w partition-broadcast on-chip: K=1 matmuls into s0 PSUM,
        # read directly by the DVE.  whu first (gates uw').
        def wcast(c):
            wt = ps.tile([P, D], f32, tag="s0")
            nc.tensor.matmul(
                wt[:], ones_row[:], aw_sb[:, ds(c * D, D)], start=True, stop=True
            )
            return wt

        # ---- hT via PE transposes (f32r moving ident), cast-evict to bf16.
        hT16 = consts.tile([P, KC * JX], bf16)    # chunk k: hT[k*128+p, i]
        hT3 = hT16[:].rearrange("p (k x) -> p k x", k=KC)

        def transpose_tile(t):
            pt = ps.tile([P, KC * P], f32r, tag="tp")
            for k in range(KC):
                nc.tensor.transpose(
                    pt[:, ds(k * P, P)], h_all[:, ds(t * D + k * P, P)], ident[:]
                )
            ev = nc.vector.tensor_copy if t % 2 == 0 else nc.scalar.copy
            ev(hT3[:, :, ds(t * P, P)],
               pt[:].bitcast(f32).rearrange("p (k x) -> p k x", k=KC))

        transpose_tile(0)
        whu_p = wcast(2)
        wh_p = wcast(0)
        transpose_tile(1)
        wu_p = wcast(1)

        # uw[j,d] = u[j,d]*w_hu[d] + w_h[d];  uwu[j] = sum_d u[j,d]*w_u[d]
        uw = consts.tile([JQ, D], f32r)
        uw0 = consts.tile([JQ, D], f32)
        nc.vector.tensor_mul(uw0[:], u_f, whu_p[:])
        nc.vector.tensor_add(uw[:], uw0[:], wh_p[:])
        uwtmp = consts.tile([JQ, D], f32)
        uwu = consts.tile([JQ, 1], f32)
        nc.vector.scalar_tensor_tensor(
            uwtmp[:], u_f, 1.0, wu_p[:],
            op0=MUL, op1=MUL, accum_out=uwu[:],
        )

        transpose_tile(2)

        # uwT16[d_chunk][j]: 4 transposes into one PSUM bank, one cast-evict
        uwT16 = consts.tile([P, KC * JQ], bf16)
        ptw = ps.tile([P, KC * P], f32r, tag="tp")
        for k in range(KC):
            nc.tensor.transpose(ptw[:, ds(k * P, P)], uw[:, ds(k * P, P)], ident[:])
        nc.scalar.copy(uwT16[:], ptw[:].bitcast(f32))

        transpose_tile(3)

        # ---- scores (transposed layout): sT[j,i] over a 512-wide i-block
        ET16 = consts.tile([JQ, JX], bf16)        # exp(sT + uwu[j]) = exp(s - b)
        m16 = consts.tile([P, NT], f32r)          # per i-tile: max_j ET (f32r: hap pairs with f32r h_all)
        z_rec = consts.tile([P, NT], f32)         # per i-tile: 1/sum_j ET

        def block_scores(b):
            sp = ps.tile([JQ, IB], f32, tag="s0")
            for k in range(KC):
                nc.tensor.matmul(
                    sp[:], uwT16[:, ds(k * JQ, JQ)], hT3[:, k, ds(b * IB, IB)],
                    start=(k == 0), stop=(k == KC - 1),
                )
            # ET = exp(sT + uwu[j]); uwu is the per-partition (j) ACT bias
            nc.scalar.activation(ET16[:, ds(b * IB, IB)], sp[:], EXP, bias=uwu[:])

        mrow = consts.tile([P, 1], f32)

        def block_reduce(b):
            # re-transpose ET (4 tiles into one bank, bf16); batched reduces.
            # For block 1, mrow (the q2c chain head) is squeezed between MAX
            # and the zsum so the hap/bc path isn't queued behind it.
            et = ps.tile([P, TPB * P], bf16, tag="tp")
            for q in range(TPB):
                t = b * TPB + q
                nc.tensor.transpose(
                    et[:, ds(q * P, P)], ET16[:, ds(t * P, P)], ident16[:]
                )
            et3 = et[:].rearrange("p (q x) -> p q x", q=TPB)
            nc.vector.reduce_max(m16[:, ds(b * TPB, TPB)], et3, axis=AX)
            if b == 1:
                nc.vector.reduce_sum(mrow[:], m16[:].bitcast(f32), axis=AX)
            zsum = stage.tile([P, TPB], f32, tag="zs")
            nc.vector.reduce_sum(zsum[:], et3, axis=AX)
            nc.vector.reciprocal(z_rec[:, ds(b * TPB, TPB)], zsum[:])

        block_scores(0)
        transpose_tile(4)
        transpose_tile(5)
        transpose_tile(6)
        transpose_tile(7)
        block_reduce(0)
        block_scores(1)

        hap = ps.tile([1, D], f32, tag="hap", bufs=1)

        def hap_block(b):
            # q2c accumulation (single PSUM group spanning both blocks)
            for q in range(TPB):
                t = b * TPB + q
                nc.tensor.matmul(
                    hap[:], m16[:, ds(t, 1)], h_all[:, ds(t * D, D)],
                    start=(b == 0 and q == 0), stop=(b == NB - 1 and q == TPB - 1),
                    skip_group_check=True,
                )

        # ---- c2q: up = unnormalized a_u @ u per tile; col1 = up*zr (per-tile
        # scalar, ACT evens / DVE odds), col2 = col1*h as 2-wide SBUF muls.
        stgA = consts.tile([P, NT * D], bf16)   # u_a        (out123 col 0)
        stgB = consts.tile([P, NT * D], bf16)   # h * u_a    (out123 col 1)
        o4_16 = consts.tile([P, NT * D], bf16)  # h * h_a    (out123 col 2)

        ups = {}

        def c2q_tile(t):
            up = ps.tile([P, D], f32, tag="ua")
            ups[t] = up
            nc.tensor.matmul(
                up[:], ET16[:, ds(t * P, P)], u16[:], start=True, stop=True
            )
            # col1 = up * zr on ACT (frees DVE for col2/o4)
            nc.scalar.mul(stgA[:, ds(t * D, D)], ups[t][:], z_rec[:, ds(t, 1)])

        def col2_tile(t):
            # col2 = (up * zr) * h as one f32-in DVE stt (mixed bf16/f32
            # tensor_tensor on DVE hits a 4x-slow path; this doesn't)
            nc.vector.scalar_tensor_tensor(
                stgB[:, ds(t * D, D)], ups[t][:], z_rec[:, ds(t, 1)],
                h_f[:, ds(t * D, D)], op0=MUL, op1=MUL,
            )

        def col2_pair_gps(t0):
            # GpSimd 2-wide from SBUF (needs col1 done for both tiles)
            nc.gpsimd.tensor_mul(
                stgB[:, ds(t0 * D, 2 * D)],
                stgA[:, ds(t0 * D, 2 * D)],
                h_f[:, ds(t0 * D, 2 * D)],
            )

        def stg_writes(half):
            t0 = half * TPB
            for col, src in ((0, stgA), (1, stgB)):
                nc.sync.dma_start(
                    out123_d[ds(t0 * P, TPB * P), ds(col * D, D)]
                    .rearrange("(t p) c -> p t c", p=P),
                    src[:, ds(t0 * D, TPB * D)].rearrange("p (t c) -> p t c", t=TPB),
                )

        # ---- spine: c2q t0-3 in the EXP-1/reduce-1 shadow; hap prioritized
        # on PE; reduces-b1 + mrow + rzq emitted on DVE before the bulk col2
        # muls so the q2c chain isn't head-of-line blocked.
        c2q_tile(0)
        c2q_tile(1)
        block_reduce(1)
        col2_pair_gps(0)
        c2q_tile(2)
        c2q_tile(3)
        col2_pair_gps(2)
        hap_block(0)
        hap_block(1)

        # q2c normalization chain: rzq = 1/sum_i m_exp[i] folds into the
        # ha_row eviction as an ACT scale, so bc is the NORMALIZED h_a
        # broadcast and o4 is a plain tensor_mul.
        zqp = ps.tile([1, 1], f32, tag="acc", bufs=1)
        nc.tensor.matmul(zqp[:], mrow[:], ones_col[:], start=True, stop=True)
        rzq = consts.tile([1, 1], f32)
        nc.vector.reciprocal(rzq[:], zqp[:])
        ha_row = consts.tile([1, D], bf16)
        nc.scalar.mul(ha_row[:], hap[:], rzq[:])
        bc = ps.tile([P, D], f32, tag="acc", bufs=1)
        nc.tensor.matmul(bc[:], ones_row16[:], ha_row[:], start=True, stop=True)
        bc_sb = consts.tile([P, D], bf16)
        nc.scalar.copy(bc_sb[:], bc[:])

        bc2 = bc[:].rearrange("p (o d) -> p o d", o=1).broadcast_to([P, 2, D])
        bc_sb2 = bc_sb[:].rearrange("p (o d) -> p o d", o=1).broadcast_to([P, 2, D])

        def o4_pair(t0, eng):
            src = bc2 if eng is nc.vector else bc_sb2
            eng.tensor_mul(
                o4_16[:, ds(t0 * D, 2 * D)].rearrange("p (t d) -> p t d", t=2),
                h_f[:, ds(t0 * D, 2 * D)].rearrange("p (t d) -> p t d", t=2),
                src,
            )

        def o4_write(half):
            t0 = half * TPB
            nc.sync.dma_start(
                out123_d[ds(t0 * P, TPB * P), ds(2 * D, D)]
                .rearrange("(t p) c -> p t c", p=P),
                o4_16[:, ds(t0 * D, TPB * D)].rearrange("p (t c) -> p t c", t=TPB),
            )

        c2q_tile(4)
        c2q_tile(5)
        stg_writes(0)
        col2_tile(4)
        c2q_tile(6)
        col2_tile(5)
        c2q_tile(7)
        col2_tile(6)
        col2_tile(7)
        o4_pair(0, nc.vector)
        o4_pair(4, nc.gpsimd)
        o4_pair(2, nc.vector)
        o4_write(0)
        stg_writes(1)
        o4_pair(6, nc.vector)
        nc.sync.dma_start(
            out123_d[ds(4 * P, 2 * P), ds(2 * D, D)]
            .rearrange("(t p) c -> p t c", p=P),
            o4_16[:, ds(4 * D, 2 * D)].rearrange("p (t c) -> p t c", t=2),
        )
        nc.sync.dma_start(
            out123_d[ds(6 * P, 2 * P), ds(2 * D, D)]
            .rearrange("(t p) c -> p t c", p=P),
            o4_16[:, ds(6 * D, 2 * D)].rearrange("p (t c) -> p t c", t=2),
        )

    nc.compile()
    return nc


def _get_nc():
    if "nc" not in _CACHE:
        _CACHE["nc"] = _build_program()
    return _CACHE["nc"]


def _ensure_axon_hooks_stub():
    # concourse imports antenv.axon_hooks when tracing is requested via env;
    # provide a no-op stub if the image lacks it so runs degrade gracefully.
    import sys
    import types

    try:
        import antenv.axon_hooks  # noqa: F401
    except ImportError:
        mod = types.ModuleType("antenv.axon_hooks")
        _hook = [None]
        mod.set_axon_ntff_profile_hook = lambda hook: _hook.__setitem__(0, hook)
        mod.get_axon_ntff_profile_hook = lambda: _hook[0]
        sys.modules["antenv.axon_hooks"] = mod


def _postprocess(res):
    out = np.empty((N_B, JX, 4 * D), dtype=np.float32)
    for n in range(N_B):
        out[n, :, :D] = np.asarray(res.results[n]["out0"])
        out[n, :, D:] = np.asarray(res.results[n]["out123"]).astype(np.float32)
    return out.reshape(N_B, M_B, JX, 4 * D)


def kernel(h, u, alpha_w, alpha_b=None, **_unused):
    _ensure_axon_hooks_stub()
    from concourse.bass_utils import run_bass_kernel_spmd

    h = np.ascontiguousarray(np.asarray(h, dtype=np.float32)).reshape(N_B, JX, D)
    u = np.ascontiguousarray(np.asarray(u, dtype=np.float32)).reshape(N_B, JQ, D)
    alpha_w = np.ascontiguousarray(np.asarray(alpha_w, dtype=np.float32)).reshape(3 * D)

    nc = _get_nc()
    in_maps = [
        {"h": h[n], "u": u[n], "alpha_w": alpha_w} for n in range(N_B)
    ]
    res = run_bass_kernel_spmd(nc, in_maps, core_ids=list(range(N_B)))
    return _postprocess(res)


# revision 12
# speedup vs baseline: 1.0586x; 1.0036x over previous
"""Trainium2 Bass kernel for BiDAF-style bidirectional attention.

Reference computation (per batch element n; M=1 folded away):
    s[i,j]  = h[i].w_h + u[j].w_u + (h[i]*u[j]).w_hu + b      [JX, JQ]
    a_u     = softmax_j(s);     u_a[i] = sum_j a_u[i,j] u[j]   (c2q)
    a_h     = softmax_i(max_j s);  h_a = sum_i a_h[i] h[i]     (q2c)
    out     = concat(h, u_a, h*u_a, h*h_a)                     [JX, 4D]

Sharding: data-parallel over batch N=8, one NeuronCore per batch element.
alpha_b drops out entirely (both softmaxes are shift-invariant).

v3 structure (51us f32 baseline -> target ~30us):
  - output split in DRAM: out0 = f32 [JX, D] passthrough column, written by
    a single DRAM->DRAM DMA of h issued at kernel start (flows during the
    otherwise-idle init window, no SBUF round trip, no cast); out123 = bf16
    [JX, 3D] for the three computed columns (store traffic 6MB -> 3MB, the
    2e-2 rel-err gate leaves ~5x margin over bf16 rounding).
  - bf16 matmul operands (hT, ET, u, uw', m_exp): PE moving cost 1 cyc/col
    instead of 1.5 (f32r), PSUM->SBUF evictions cast-on-copy to bf16.
    All accumulation stays f32 in PSUM.
  - few big DMAs (~0.6us issue + ~0.9us completion each): h loads as 2
    singles + 3 pair-tiles, output writes as 6 merged 4-tile transfers,
    all on the sync queue in readiness order.
  - PE spine: block-0 transposes -> scores-b0 -> block-1 transposes ->
    etr-b0 -> scores-b1 -> etr-b1 -> c2q/hap interleaved, so EXP-1 lands
    ~10us earlier than the baseline's schedule.
  - tail elementwise ops are 2-tile-wide (1024 cols) where legal to
    amortize the ~400ns per-instruction engine overhead; col1 (the only
    PSUM-reading mul with a per-tile scalar) stays per-tile, split
    ACT/DVE.  GpSimd (no PSUM port, ~1.6x slower) gets SBUF-only pairs.
"""

import numpy as np

N_B, M_B, JX, JQ, D = 8, 1, 1024, 128, 512
P = 128
NT = JX // P   # 8 i-tiles
KC = D // P    # 4 d-chunks
IB = 512       # i-block width for score matmuls
NB = JX // IB  # 2 blocks
TPB = NT // NB  # tiles per block

_CACHE = {}


def _build_program():
    from contextlib import ExitStack

    import concourse.bass as bass
    import concourse.tile as tile
    from concourse import bacc, mybir
    from concourse.masks import make_identity
    from concourse.tile_rust import add_dep_helper

    f32 = mybir.dt.float32
    f32r = mybir.dt.float32r
    bf16 = mybir.dt.bfloat16
    EXP = mybir.ActivationFunctionType.Exp
    AX = mybir.AxisListType.X
    MUL = mybir.AluOpType.mult
    ds = bass.ds

    nc = bacc.Bacc("TRN2", target_bir_lowering=False, debug=False, num_devices=8)
    h_d = nc.dram_tensor("h", [JX, D], f32, kind="ExternalInput").ap()
    u_d = nc.dram_tensor("u", [JQ, D], f32, kind="ExternalInput").ap()
    aw_d = nc.dram_tensor("alpha_w", [3 * D], f32, kind="ExternalInput").ap()
    out0_d = nc.dram_tensor("out0", [JX, D], f32, kind="ExternalOutput").ap()
    out123_d = nc.dram_tensor("out123", [JX, 3 * D], bf16, kind="ExternalOutput").ap()

    with tile.TileContext(nc) as tc, ExitStack() as ctx:
        consts = ctx.enter_context(tc.tile_pool(name="consts", bufs=1))
        stage = ctx.enter_context(tc.tile_pool(name="stage", bufs=4))
        # PSUM budget (8 banks): tp=2, s0=2, ua=2, acc=1, hap=1
        ps = ctx.enter_context(tc.tile_pool(name="ps", bufs=2, space="PSUM"))

        # ---- PE warmup: f32r matmuls depending only on DVE ops, emitted
        # first so the HAM clock-gate opens (1.2 -> 2.4 GHz) while the h
        # DMAs stream in.
        warm_f = consts.tile([P, D], f32)
        nc.vector.memset(warm_f[:], 0.25)
        warm = consts.tile([P, D], f32r)
        nc.vector.tensor_copy(warm[:], warm_f[:])
        wp = ps.tile([P, D], f32, tag="acc", bufs=1)
        for w in range(2):
            nc.tensor.matmul(
                wp[:], warm[:, ds(0, P)], warm[:], start=True, stop=True,
            )

        # ---- constants ----
        ident_f = consts.tile([P, P], f32)
        make_identity(nc, ident_f[:])
        ident = consts.tile([P, P], f32r)
        nc.vector.tensor_copy(ident[:], ident_f[:])
        ident16 = consts.tile([P, P], bf16)
        nc.vector.tensor_copy(ident16[:], ident_f[:])
        ones_row_f = consts.tile([1, P], f32)
        nc.vector.memset(ones_row_f[:], 1.0)
        ones_row = consts.tile([1, P], f32r)
        nc.scalar.copy(ones_row[:], ones_row_f[:])
        ones_row16 = consts.tile([1, P], bf16)
        nc.scalar.copy(ones_row16[:], ones_row_f[:])
        ones_col = consts.tile([P, 1], f32)
        nc.vector.memset(ones_col[:], 1.0)

        # ---- loads (sync queue): u + aw first (they feed the uw' prep
        # chain), then h tiles 0,1 single (they gate the first transposes)
        # and 2-7 as pairs.
        h_all = consts.tile([P, NT * D], f32r)    # tile t: h[t*128+p, d]
        h_f = h_all[:].bitcast(f32)
        for t in range(2):
            nc.sync.dma_start(
                h_all[:, ds(t * D, D)], h_d[ds(t * P, P), :].bitcast(f32r)
            )
        aw_sb = consts.tile([1, 3 * D], f32r)
        nc.sync.dma_start(aw_sb[:], aw_d.rearrange("(o d) -> o d", o=1).bitcast(f32r))
        u_sb = consts.tile([JQ, D], f32r)
        nc.sync.dma_start(u_sb[:], u_d[:].bitcast(f32r))
        u_f = u_sb[:].bitcast(f32)
        last_h = None
        for t in range(2, NT):
            last_h = nc.sync.dma_start(
                h_all[:, ds(t * D, D)], h_d[ds(t * P, P), :].bitcast(f32r)
            )

        # passthrough column: DRAM->DRAM copy of h in 8KB rows.  Gated on the
        # last h load: its big descriptors win the per-packet round-robin, so
        # letting it overlap the loads starves them (~150GB/s effective).  In
        # the post-load lull it's free.
        d2d = nc.sync.dma_start(
            out0_d.rearrange("(a b) d -> a (b d)", b=4),
            h_d.rearrange("(a b) d -> a (b d)", b=4),
        )
        add_dep_helper(d2d.ins, last_h.ins, sync=True,
                       reason="keep d2d pass copy out of the load window")

        u16 = consts.tile([JQ, D], bf16)
        nc.gpsimd.tensor_copy(u16[:], u_f)

        # alpha_w partition-broadcast on-chip: K=1 matmuls into s0 PSUM,
        # read directly by the DVE.  whu first (gates uw').
        def wcast(c):
            wt = ps.tile([P, D], f32, tag="s0")
            nc.tensor.matmul(
                wt[:], ones_row[:], aw_sb[:, ds(c * D, D)], start=True, stop=True
            )
            return wt

        # ---- hT via PE transposes (f32r moving ident), cast-evict to bf16.
        hT16 = consts.tile([P, KC * JX], bf16)    # chunk k: hT[k*128+p, i]
        hT3 = hT16[:].rearrange("p (k x) -> p k x", k=KC)

        def transpose_tile(t):
            pt = ps.tile([P, KC * P], f32r, tag="tp")
            for k in range(KC):
                nc.tensor.transpose(
                    pt[:, ds(k * P, P)], h_all[:, ds(t * D + k * P, P)], ident[:]
                )
            ev = nc.vector.tensor_copy if t % 2 == 0 else nc.scalar.copy
            ev(hT3[:, :, ds(t * P, P)],
               pt[:].bitcast(f32).rearrange("p (k x) -> p k x", k=KC))

        transpose_tile(0)
        whu_p = wcast(2)
        wh_p = wcast(0)
        transpose_tile(1)
        wu_p = wcast(1)

        # uw[j,d] = u[j,d]*w_hu[d] + w_h[d];  uwu[j] = sum_d u[j,d]*w_u[d]
        uw = consts.tile([JQ, D], f32r)
        uw0 = consts.tile([JQ, D], f32)
        nc.vector.tensor_mul(uw0[:], u_f, whu_p[:])
        nc.vector.tensor_add(uw[:], uw0[:], wh_p[:])
        uwtmp = consts.tile([JQ, D], f32)
        uwu = consts.tile([JQ, 1], f32)
        nc.vector.scalar_tensor_tensor(
            uwtmp[:], u_f, 1.0, wu_p[:],
            op0=MUL, op1=MUL, accum_out=uwu[:],
        )

        transpose_tile(2)

        # uwT16[d_chunk][j]: 4 transposes into one PSUM bank, one cast-evict
        uwT16 = consts.tile([P, KC * JQ], bf16)
        ptw = ps.tile([P, KC * P], f32r, tag="tp")
        for k in range(KC):
            nc.tensor.transpose(ptw[:, ds(k * P, P)], uw[:, ds(k * P, P)], ident[:])
        nc.scalar.copy(uwT16[:], ptw[:].bitcast(f32))

        transpose_tile(3)

        # ---- scores (transposed layout): sT[j,i] over a 512-wide i-block
        ET16 = consts.tile([JQ, JX], bf16)        # exp(sT + uwu[j]) = exp(s - b)
        m16 = consts.tile([P, NT], f32r)          # per i-tile: max_j ET (f32r: hap pairs with f32r h_all)
        z_rec = consts.tile([P, NT], f32)         # per i-tile: 1/sum_j ET

        def block_scores(b):
            sp = ps.tile([JQ, IB], f32, tag="s0")
            for k in range(KC):
                nc.tensor.matmul(
                    sp[:], uwT16[:, ds(k * JQ, JQ)], hT3[:, k, ds(b * IB, IB)],
                    start=(k == 0), stop=(k == KC - 1),
                )
            # ET = exp(sT + uwu[j]); uwu is the per-partition (j) ACT bias
            nc.scalar.activation(ET16[:, ds(b * IB, IB)], sp[:], EXP, bias=uwu[:])

        mrow = consts.tile([P, 1], f32)

        def block_reduce(b):
            # re-transpose ET (4 tiles into one bank, bf16); batched reduces.
            # For block 1, mrow (the q2c chain head) is squeezed between MAX
            # and the zsum so the hap/bc path isn't queued behind it.
            et = ps.tile([P, TPB * P], bf16, tag="tp")
            for q in range(TPB):
                t = b * TPB + q
                nc.tensor.transpose(
                    et[:, ds(q * P, P)], ET16[:, ds(t * P, P)], ident16[:]
                )
            et3 = et[:].rearrange("p (q x) -> p q x", q=TPB)
            nc.vector.reduce_max(m16[:, ds(b * TPB, TPB)], et3, axis=AX)
            if b == 1:
                nc.vector.reduce_sum(mrow[:], m16[:].bitcast(f32), axis=AX)
            zsum = stage.tile([P, TPB], f32, tag="zs")
            nc.vector.reduce_sum(zsum[:], et3, axis=AX)
            nc.vector.reciprocal(z_rec[:, ds(b * TPB, TPB)], zsum[:])

        block_scores(0)
        transpose_tile(4)
        transpose_tile(5)
        transpose_tile(6)
        transpose_tile(7)
        block_reduce(0)
        block_scores(1)

        hap = ps.tile([1, D], f32, tag="hap", bufs=1)

        def hap_block(b):
            # q2c accumulation (single PSUM group spanning both blocks)
            for q in range(TPB):
                t = b * TPB + q
                nc.tensor.matmul(
                    hap[:], m16[:, ds(t, 1)], h_all[:, ds(t * D, D)],
                    start=(b == 0 and q == 0), stop=(b == NB - 1 and q == TPB - 1),
                    skip_group_check=True,
                )

        # ---- c2q: up = unnormalized a_u @ u per tile; col1 = up*zr (per-tile
        # scalar, ACT evens / DVE odds), col2 = col1*h as 2-wide SBUF muls.
        stgA = consts.tile([P, NT * D], bf16)   # u_a        (out123 col 0)
        stgB = consts.tile([P, NT * D], bf16)   # h * u_a    (out123 col 1)
        o4_16 = consts.tile([P, NT * D], bf16)  # h * h_a    (out123 col 2)

        ups = {}

        def c2q_tile(t):
            up = ps.tile([P, D], f32, tag="ua")
            ups[t] = up
            nc.tensor.matmul(
                up[:], ET16[:, ds(t * P, P)], u16[:], start=True, stop=True
            )
            # col1 = up * zr on ACT (frees DVE for col2/o4)
            nc.scalar.mul(stgA[:, ds(t * D, D)], ups[t][:], z_rec[:, ds(t, 1)])

        def col2_tile(t):
            # col2 = (up * zr) * h as one f32-in DVE stt (mixed bf16/f32
            # tensor_tensor on DVE hits a 4x-slow path; this doesn't)
            nc.vector.scalar_tensor_tensor(
                stgB[:, ds(t * D, D)], ups[t][:], z_rec[:, ds(t, 1)],
                h_f[:, ds(t * D, D)], op0=MUL, op1=MUL,
            )

        def col2_pair_gps(t0):
            # GpSimd 2-wide from SBUF (needs col1 done for both tiles)
            nc.gpsimd.tensor_mul(
                stgB[:, ds(t0 * D, 2 * D)],
                stgA[:, ds(t0 * D, 2 * D)],
                h_f[:, ds(t0 * D, 2 * D)],
            )

        def stg_writes(half):
            t0 = half * TPB
            for col, src in ((0, stgA), (1, stgB)):
                nc.sync.dma_start(
                    out123_d[ds(t0 * P, TPB * P), ds(col * D, D)]
                    .rearrange("(t p) c -> p t c", p=P),
                    src[:, ds(t0 * D, TPB * D)].rearrange("p (t c) -> p t c", t=TPB),
                )

        # ---- spine: c2q t0-3 in the EXP-1/reduce-1 shadow; hap prioritized
        # on PE; reduces-b1 + mrow + rzq emitted on DVE before the bulk col2
        # muls so the q2c chain isn't head-of-line blocked.
        c2q_tile(0)
        c2q_tile(1)
        block_reduce(1)
        col2_pair_gps(0)
        c2q_tile(2)
        c2q_tile(3)
        col2_pair_gps(2)
        hap_block(0)
        hap_block(1)

        # q2c normalization chain: rzq = 1/sum_i m_exp[i] folds into the
        # ha_row eviction as an ACT scale, so bc is the NORMALIZED h_a
        # broadcast and o4 is a plain tensor_mul.
        zqp = ps.tile([1, 1], f32, tag="acc", bufs=1)
        nc.tensor.matmul(zqp[:], mrow[:], ones_col[:], start=True, stop=True)
        rzq = consts.tile([1, 1], f32)
        nc.vector.reciprocal(rzq[:], zqp[:])
        ha_row = consts.tile([1, D], bf16)
        nc.scalar.mul(ha_row[:], hap[:], rzq[:])
        bc = ps.tile([P, D], f32, tag="acc", bufs=1)
        nc.tensor.matmul(bc[:], ones_row16[:], ha_row[:], start=True, stop=True)
        bc_sb = consts.tile([P, D], bf16)
        nc.scalar.copy(bc_sb[:], bc[:])

        bc2 = bc[:].rearrange("p (o d) -> p o d", o=1).broadcast_to([P, 2, D])
        bc_sb2 = bc_sb[:].rearrange("p (o d) -> p o d", o=1).broadcast_to([P, 2, D])

        def o4_pair(t0, eng):
            src = bc2 if eng is nc.vector else bc_sb2
            eng.tensor_mul(
                o4_16[:, ds(t0 * D, 2 * D)].rearrange("p (t d) -> p t d", t=2),
                h_f[:, ds(t0 * D, 2 * D)].rearrange("p (t d) -> p t d", t=2),
                src,
            )

        def o4_write(half):
            t0 = half * TPB
            nc.sync.dma_start(
                out123_d[ds(t0 * P, TPB * P), ds(2 * D, D)]
                .rearrange("(t p) c -> p t c", p=P),
                o4_16[:, ds(t0 * D, TPB * D)].rearrange("p (t c) -> p t c", t=TPB),
            )

        c2q_tile(4)
        c2q_tile(5)
        stg_writes(0)
        col2_tile(4)
        c2q_tile(6)
        col2_tile(5)
        c2q_tile(7)
        col2_tile(6)
        col2_tile(7)
        o4_pair(0, nc.vector)
        o4_pair(4, nc.gpsimd)
        o4_pair(2, nc.vector)
        o4_write(0)
        stg_writes(1)
        o4_pair(6, nc.vector)
        nc.sync.dma_start(
            out123_d[ds(4 * P, 2 * P), ds(2 * D, D)]
            .rearrange("(t p) c -> p t c", p=P),
            o4_16[:, ds(4 * D, 2 * D)].rearrange("p (t c) -> p t c", t=2),
        )
        nc.sync.dma_start(
            out123_d[ds(6 * P, 2 * P), ds(2 * D, D)]
            .rearrange("(t p) c -> p t c", p=P),
            o4_16[:, ds(6 * D, 2 * D)].rearrange("p (t c) -> p t c", t=2),
        )

    nc.compile()
    return nc


def _get_nc():
    if "nc" not in _CACHE:
        _CACHE["nc"] = _build_program()
    return _CACHE["nc"]


def _ensure_axon_hooks_stub():
    # concourse imports antenv.axon_hooks when tracing is requested via env;
    # provide a no-op stub if the image lacks it so runs degrade gracefully.
    import sys
    import types

    try:
        import antenv.axon_hooks  # noqa: F401
    except ImportError:
        mod = types.ModuleType("antenv.axon_hooks")
        _hook = [None]
        mod.set_axon_ntff_profile_hook = lambda hook: _hook.__setitem__(0, hook)
        mod.get_axon_ntff_profile_hook = lambda: _hook[0]
        sys.modules["antenv.axon_hooks"] = mod


def _postprocess(res):
    out = np.empty((N_B, JX, 4 * D), dtype=np.float32)
    for n in range(N_B):
        out[n, :, :D] = np.asarray(res.results[n]["out0"])
        out[n, :, D:] = np.asarray(res.results[n]["out123"]).astype(np.float32)
    return out.reshape(N_B, M_B, JX, 4 * D)


def kernel(h, u, alpha_w, alpha_b=None, **_unused):
    _ensure_axon_hooks_stub()
    from concourse.bass_utils import run_bass_kernel_spmd

    h = np.ascontiguousarray(np.asarray(h, dtype=np.float32)).reshape(N_B, JX, D)
    u = np.ascontiguousarray(np.asarray(u, dtype=np.float32)).reshape(N_B, JQ, D)
    alpha_w = np.ascontiguousarray(np.asarray(alpha_w, dtype=np.float32)).reshape(3 * D)

    nc = _get_nc()
    in_maps = [
        {"h": h[n], "u": u[n], "alpha_w": alpha_w} for n in range(N_B)
    ]
    res = run_bass_kernel_spmd(nc, in_maps, core_ids=list(range(N_B)))
    return _postprocess(res)
